# revision 1
# baseline (speedup 1.0000x reference)
"""Contrastive loss kernel for Trainium2 (8 NeuronCores, Bass/Tile).

Strategy (data-parallel over rows of embeddings1):
  - core c owns rows [c*CH, (c+1)*CH) of e1 ("i" index).
  - every core holds all of e2 (passed pre-transposed+bf16 from host) and
    computes the transposed logit tile  S_T[j, i] = <e2_j, e1n_i>  for all
    N j and its CH i's.  j lives on the partition axis, so the per-row
    scale 1/(T*||e2_j||) rides the ACT Exp `scale` vector, and the ACT
    `accum_out` gives the per-j partial column sums for free.
  - row sums (over all j) are partition-dim reductions done on the PE with
    a ones-vector stationary operand, accumulated in PSUM.
  - diagonal logits are computed separately as an exact f32 row-wise dot
    product e1n_i . e2_i (needs only the core's own CH rows of e2).
  - host combines: 8x partial colsums -> full column sums, subtracts the
    diagonal exp, takes logs and the two scalar sums.

Outputs per core: colp [128,JT] (column partial sums, j = jt*128+p),
rows [1,CH] (row sums incl. diagonal), ldiag [128,IT] (diag logits).
"""

import os
import sys

import numpy as np

for _p in ("/root/.axon_site", "/root/.axon_site/_ro/trn_rl_repo",
           "/root/.axon_site/_ro/pypackages", "/opt/trn_rl_repo"):
    if os.path.isdir(_p) and _p not in sys.path:
        sys.path.append(_p)

import ml_dtypes

N, D = 4096, 1024
NCORES = 8
CH = N // NCORES          # 512 rows of e1 per core
INV_T = 10.0              # 1 / temperature

_CACHE = {}


def _legalize_waits(nc, cap=1):
    """Split >cap semaphore waits per instruction onto preceding NOPs.

    The walrus build in this container rejects instructions carrying more
    than ~2 sync waits ("Too many sync wait commands"); Tile emits up to
    12 on the final barrier drain.  Hoisting the excess waits onto NOPs
    issued just before, on the same engine queue, is semantics-preserving
    (the engine is in-order, so waiting earlier is safe).
    """
    import concourse.mybir as mybir
    nid = 0
    for f in nc.m.functions:
        for b in f.blocks:
            insts = b.instructions
            i = 0
            while i < len(insts):
                inst = insts[i]
                si = inst.sync_info
                if si is not None and si.on_wait and len(si.on_wait) > cap:
                    waits = list(si.on_wait)
                    inst.sync_info = mybir.SyncInfo(
                        on_wait=waits[-cap:], on_update=list(si.on_update))
                    excess = waits[:-cap]
                    pos = i
                    for j in range(0, len(excess), cap):
                        nop = mybir.InstNoOp(
                            name=f"I-waitnop-{nid}", ins=[], outs=[])
                        nid += 1
                        nop.engine = inst.engine
                        nop.sync_info = mybir.SyncInfo(
                            on_wait=excess[j:j + cap], on_update=[])
                        insts.insert(pos, nop)
                        pos += 1
                        i += 1
                i += 1
    return nc


def build_nc(n=N, d=D, ch=CH, legalize=True):
    import concourse.bass as bass
    import concourse.mybir as mybir
    import concourse.tile as tile
    from concourse.masks import make_identity
    from contextlib import ExitStack

    fp32 = mybir.dt.float32
    bf16 = mybir.dt.bfloat16
    AF = mybir.ActivationFunctionType
    OP = mybir.AluOpType
    ts = bass.ts

    kt = d // 128             # contraction tiles
    jt_n = n // 128           # j tiles
    it_n = ch // 128          # i tiles

    nc = bass.Bass(trn_type="TRN2")
    e1c_d = nc.dram_tensor("e1c", [ch, d], fp32, kind="ExternalInput")
    e2c_d = nc.dram_tensor("e2c", [ch, d], fp32, kind="ExternalInput")
    e2t_d = nc.dram_tensor("e2t", [d, n], bf16, kind="ExternalInput")
    e2r_d = nc.dram_tensor("e2r", [n, d], bf16, kind="ExternalInput")
    colp_d = nc.dram_tensor("colp", [128, jt_n], fp32, kind="ExternalOutput")
    rows_d = nc.dram_tensor("rows", [1, ch], fp32, kind="ExternalOutput")
    ldiag_d = nc.dram_tensor("ldiag", [128, it_n], fp32, kind="ExternalOutput")

    with ExitStack() as ctx:
        tc = ctx.enter_context(tile.TileContext(nc))
        res = ctx.enter_context(tc.tile_pool(name="res", bufs=1))
        loadp = ctx.enter_context(tc.tile_pool(name="loadp", bufs=3))
        junkp = ctx.enter_context(tc.tile_pool(name="junkp", bufs=3))
        smallp = ctx.enter_context(tc.tile_pool(name="smallp", bufs=4))
        e1np = ctx.enter_context(tc.tile_pool(name="e1np", bufs=2))
        pml = ctx.enter_context(tc.tile_pool(name="pml", bufs=4, space="PSUM"))
        ptr = ctx.enter_context(tc.tile_pool(name="ptr", bufs=2, space="PSUM"))
        prow = ctx.enter_context(tc.tile_pool(name="prow", bufs=1, space="PSUM"))

        # resident SBUF tensors
        e2t_sb = res.tile([128, kt, n], bf16)     # e2^T, d on partitions
        e1t_sb = res.tile([128, kt, ch], bf16)    # normalized e1^T
        exps_sb = res.tile([128, jt_n, ch], bf16)  # exp(logits^T)
        e1f_all = res.tile([128, it_n, d], fp32)  # raw e1 rows (f32)
        colp_sb = res.tile([128, jt_n], fp32)
        ss2_sb = res.tile([128, jt_n], fp32)      # sumsq of all e2 rows
        srow = res.tile([128, jt_n], fp32)        # 10 / ||e2_j||
        norm2 = res.tile([128, jt_n], fp32)
        ldiag_sb = res.tile([128, it_n], fp32)
        ss1 = res.tile([128, it_n], fp32)
        ssc = res.tile([128, it_n], fp32)
        raw = res.tile([128, it_n], fp32)
        r1 = res.tile([128, it_n], fp32)
        rc = res.tile([128, it_n], fp32)
        rows_sb = res.tile([1, ch], fp32)
        ident = res.tile([128, 128], bf16)
        ones_bf = res.tile([128, 1], bf16)

        make_identity(nc, ident)
        nc.vector.memset(ones_bf, 1.0)

        # ---- load e2^T (stationary operand of the big matmul) ----
        for k in range(kt):
            nc.sync.dma_start(out=e2t_sb[:, k, :], in_=e2t_d[ts(k, 128), :])

        # ---- e1 rows: sumsq, diag dot with e2 rows ----
        for t in range(it_n):
            nc.sync.dma_start(out=e1f_all[:, t, :], in_=e1c_d[ts(t, 128), :])
        for t in range(it_n):
            e2f = loadp.tile([128, d], fp32, tag="e2f")
            nc.sync.dma_start(out=e2f, in_=e2c_d[ts(t, 128), :])
            junkc = junkp.tile([128, d], fp32, tag="junkc")
            nc.scalar.activation(out=junkc, in_=e1f_all[:, t, :],
                                 func=AF.Square, accum_out=ss1[:, t:t + 1])
            junkd = junkp.tile([128, d], fp32, tag="junkc")
            nc.scalar.activation(out=junkd, in_=e2f, func=AF.Square,
                                 accum_out=ssc[:, t:t + 1])
            junke = junkp.tile([128, d], fp32, tag="junkc")
            nc.vector.tensor_mul(out=junke, in0=e1f_all[:, t, :], in1=e2f)
            nc.vector.reduce_sum(out=raw[:, t:t + 1], in_=junke,
                                 axis=mybir.AxisListType.X)

        def rsqrt_nr(dst, ss):
            # dst = 1/sqrt(ss), Newton-refined to fp32 accuracy
            a = smallp.tile([128, it_n], fp32, tag="nr_a")
            nc.scalar.activation(out=a, in_=ss, func=AF.Ln)
            nc.scalar.activation(out=dst, in_=a, func=AF.Exp, scale=-0.5)
            b = smallp.tile([128, it_n], fp32, tag="nr_b")
            nc.vector.tensor_mul(out=b, in0=dst, in1=dst)
            nc.vector.tensor_mul(out=b, in0=b, in1=ss)
            nc.vector.tensor_scalar(out=b, in0=b, scalar1=-0.5, scalar2=1.5,
                                    op0=OP.mult, op1=OP.add)
            nc.vector.tensor_mul(out=dst, in0=dst, in1=b)

        rsqrt_nr(r1, ss1)
        rsqrt_nr(rc, ssc)
        # ldiag = raw * r1 * rc * 10
        m = smallp.tile([128, it_n], fp32, tag="nr_m")
        nc.vector.tensor_mul(out=m, in0=r1, in1=rc)
        nc.vector.tensor_mul(out=ldiag_sb, in0=raw, in1=m)
        nc.vector.tensor_scalar_mul(out=ldiag_sb, in0=ldiag_sb, scalar1=INV_T)
        nc.sync.dma_start(out=ldiag_d[:, :], in_=ldiag_sb)

        # ---- normalized e1 -> bf16 -> transpose onto e1t_sb ----
        for t in range(it_n):
            e1n = e1np.tile([128, d], bf16, tag="e1n")
            nc.vector.tensor_scalar_mul(out=e1n, in0=e1f_all[:, t, :],
                                        scalar1=r1[:, t:t + 1])
            for k in range(kt):
                ptile = ptr.tile([128, 128], bf16, tag="ptile")
                nc.tensor.transpose(out=ptile, in_=e1n[:, ts(k, 128)],
                                    identity=ident)
                nc.vector.tensor_copy(out=e1t_sb[:, k, ts(t, 128)], in_=ptile)

        # ---- sumsq of every e2 row ----
        # alternate engines: ACT Square(+accum) / GPSIMD square + DVE reduce
        for jt in range(jt_n):
            e2rt = loadp.tile([128, d], bf16, tag="e2rt")
            nc.sync.dma_start(out=e2rt, in_=e2r_d[ts(jt, 128), :])
            if jt % 2 == 0:
                junka = junkp.tile([128, d], bf16, tag="junka")
                nc.scalar.activation(out=junka, in_=e2rt, func=AF.Square,
                                     accum_out=ss2_sb[:, jt:jt + 1])
            else:
                junkb = junkp.tile([128, d], bf16, tag="junkb")
                nc.gpsimd.tensor_mul(out=junkb, in0=e2rt, in1=e2rt)
                nc.vector.reduce_sum(out=ss2_sb[:, jt:jt + 1], in_=junkb,
                                     axis=mybir.AxisListType.X)
        # srow = 10 / sqrt(ss2), in groups of 8 j-tiles to unblock the pipe
        g = 8 if jt_n % 8 == 0 else jt_n
        for j0 in range(0, jt_n, g):
            sl = slice(j0, j0 + g)
            nc.scalar.activation(out=norm2[:, sl], in_=ss2_sb[:, sl],
                                 func=AF.Ln)
            nc.scalar.activation(out=srow[:, sl], in_=norm2[:, sl],
                                 func=AF.Exp, scale=-0.5)
            nc.vector.tensor_scalar_mul(out=srow[:, sl], in0=srow[:, sl],
                                        scalar1=INV_T)

        # ---- main loop: 128-row j blocks of the transposed logit tile ----
        for jt in range(jt_n):
            pl = pml.tile([128, ch], fp32, tag="pl")
            for k in range(kt):
                nc.tensor.matmul(pl, lhsT=e2t_sb[:, k, ts(jt, 128)],
                                 rhs=e1t_sb[:, k, :],
                                 start=(k == 0), stop=(k == kt - 1))
            nc.scalar.activation(out=exps_sb[:, jt, :], in_=pl, func=AF.Exp,
                                 scale=srow[:, jt:jt + 1],
                                 accum_out=colp_sb[:, jt:jt + 1])

        # ---- row sums: ones^T @ exps, accumulated over all j tiles ----
        prow_t = prow.tile([1, ch], fp32)
        for jt in range(jt_n):
            nc.tensor.matmul(prow_t, lhsT=ones_bf, rhs=exps_sb[:, jt, :],
                             start=(jt == 0), stop=(jt == jt_n - 1))
        nc.scalar.copy(out=rows_sb, in_=prow_t)

        nc.sync.dma_start(out=rows_d[:, :], in_=rows_sb)
        nc.sync.dma_start(out=colp_d[:, :], in_=colp_sb)
    return _legalize_waits(nc) if legalize else nc


def _get_nc():
    if "nc" not in _CACHE:
        _CACHE["nc"] = build_nc()
    return _CACHE["nc"]


def _run(in_maps, trace=False, **kw):
    from concourse.bass_utils import run_bass_kernel_spmd
    return run_bass_kernel_spmd(_get_nc(), in_maps,
                                core_ids=list(range(NCORES)),
                                trace=trace, **kw)


def kernel(embeddings1, embeddings2, _trace=False, _full_result=False):
    e1 = np.ascontiguousarray(np.asarray(embeddings1, dtype=np.float32))
    e2 = np.ascontiguousarray(np.asarray(embeddings2, dtype=np.float32))
    assert e1.shape == (N, D) and e2.shape == (N, D)
    bf = ml_dtypes.bfloat16
    e2_bf = e2.astype(bf)
    e2t = np.ascontiguousarray(e2_bf.T)

    in_maps = []
    for c in range(NCORES):
        sl = slice(c * CH, (c + 1) * CH)
        in_maps.append({
            "e1c": np.ascontiguousarray(e1[sl]),
            "e2c": np.ascontiguousarray(e2[sl]),
            "e2t": e2t,
            "e2r": e2_bf,
        })
    bres = _run(in_maps, trace=_trace)
    outs = bres.results

    ldiag = np.concatenate(
        [np.asarray(o["ldiag"], dtype=np.float64).T.reshape(-1) for o in outs])
    rows = np.concatenate(
        [np.asarray(o["rows"], dtype=np.float64).reshape(-1) for o in outs])
    colsum = np.zeros(N, dtype=np.float64)
    for o in outs:
        colsum += np.asarray(o["colp"], dtype=np.float64).T.reshape(-1)

    ed = np.exp(ldiag)
    row_denom = rows - ed
    col_denom = colsum - ed
    sim12 = float(np.sum(ldiag - np.log(row_denom)))
    sim21 = float(np.sum(ldiag - np.log(col_denom)))
    result = (np.float32(-sim12), np.float32(-sim21))
    if _full_result:
        return result, bres
    return result



# revision 5
# speedup vs baseline: 2.5355x; 2.5355x over previous
"""Contrastive loss kernel for Trainium2 (8 NeuronCores, Bass/Tile).

Strategy (data-parallel over rows of embeddings1, fp8 DoubleRow matmul):
  - Host L2-normalizes both embedding sets and ships fp8_e4m3 transposed
    operands: e1t [D, CH] (core's own 512 rows of e1, d-major) and
    e2t [D, N] (all of e2, d-major, ROTATED by c*CH columns so each
    core's diagonal block lands in its first 4 j-tiles -- keeps the
    kernel SPMD-identical across cores).
  - Device: 32 j-tiles of the transposed logit tile S_T[j, i] (j on
    partitions, 512 i's on free) via fp8 DoubleRow matmuls (2 k-subtiles
    per instruction, 2x PE throughput vs bf16).
  - ACT applies exp(10*s) -> exps (fp8), Pool engine reduces each tile
    over free axis -> per-j column partials, PE ones-matmul over fp8
    exps -> per-i row sums, DVE extracts the diagonal logits from the
    first 4 PSUM tiles (identity-mask multiply + reduce).
  - Host combines: un-rotate column partials, ed = exp(10*ldiag),
    row/col denominators, logs, two scalar sums.

Outputs per core: colp [128,32] (column partials, local j = jt*128+p),
rows [1,512] (row sums incl. diagonal), ldiag [128,4] (diag cosines).
"""

import os
import sys

import numpy as np

for _p in ("/root/.axon_site", "/root/.axon_site/_ro/trn_rl_repo",
           "/root/.axon_site/_ro/pypackages", "/opt/trn_rl_repo"):
    if os.path.isdir(_p) and _p not in sys.path:
        sys.path.append(_p)

import ml_dtypes

N, D = 4096, 1024
NCORES = 8
CH = N // NCORES          # 512 rows of e1 per core
INV_T = 10.0              # 1 / temperature
FP8 = ml_dtypes.float8_e4m3

_CACHE = {}


def _legalize_waits(nc, cap=1):
    """Split >cap semaphore waits per instruction onto preceding NOPs.

    The walrus build in this container rejects instructions carrying more
    than ~2 sync waits ("Too many sync wait commands"); Tile emits up to
    12 on the final barrier drain.  Hoisting the excess waits onto NOPs
    issued just before, on the same engine queue, is semantics-preserving
    (the engine is in-order, so waiting earlier is safe).
    """
    import concourse.mybir as mybir
    nid = 0
    for f in nc.m.functions:
        for b in f.blocks:
            insts = b.instructions
            i = 0
            while i < len(insts):
                inst = insts[i]
                si = inst.sync_info
                if si is not None and si.on_wait and len(si.on_wait) > cap:
                    waits = list(si.on_wait)
                    inst.sync_info = mybir.SyncInfo(
                        on_wait=waits[-cap:], on_update=list(si.on_update))
                    excess = waits[:-cap]
                    pos = i
                    for j in range(0, len(excess), cap):
                        nop = mybir.InstNoOp(
                            name=f"I-waitnop-{nid}", ins=[], outs=[])
                        nid += 1
                        nop.engine = inst.engine
                        nop.sync_info = mybir.SyncInfo(
                            on_wait=excess[j:j + cap], on_update=[])
                        insts.insert(pos, nop)
                        pos += 1
                        i += 1
                i += 1
    return nc


def build_nc(n=N, d=D, ch=CH, legalize=True):
    import concourse.bass as bass
    import concourse.mybir as mybir
    import concourse.tile as tile
    from concourse.masks import make_identity
    from contextlib import ExitStack

    fp32 = mybir.dt.float32
    f8 = mybir.dt.float8e4
    AF = mybir.ActivationFunctionType
    PM = mybir.MatmulPerfMode.DoubleRow
    ts = bass.ts

    kt = d // 128             # 8 contraction subtiles
    jt_n = n // 128           # 32 j tiles
    it_n = ch // 128          # 4 i tiles

    nc = bass.Bass(trn_type="TRN2")
    e1t_d = nc.dram_tensor("e1t", [d, ch], f8, kind="ExternalInput")
    e2t_d = nc.dram_tensor("e2t", [d, n], f8, kind="ExternalInput")
    colp_d = nc.dram_tensor("colp", [128, jt_n], fp32, kind="ExternalOutput")
    rows_d = nc.dram_tensor("rows", [1, ch], fp32, kind="ExternalOutput")
    ldiag_d = nc.dram_tensor("ldiag", [128, it_n], fp32, kind="ExternalOutput")

    with ExitStack() as ctx:
        tc = ctx.enter_context(tile.TileContext(nc))
        res = ctx.enter_context(tc.tile_pool(name="res", bufs=1))
        junkp = ctx.enter_context(tc.tile_pool(name="junkp", bufs=2))
        pml = ctx.enter_context(tc.tile_pool(name="pml", bufs=6, space="PSUM"))
        prow = ctx.enter_context(tc.tile_pool(name="prow", bufs=1, space="PSUM"))

        # resident SBUF tensors
        e2t_sb = res.tile([128, kt, n], f8)       # e2n^T, d on partitions
        e1t_sb = res.tile([128, kt, ch], f8)      # e1n^T
        exps_sb = res.tile([128, jt_n, ch], f8)   # exp(logits^T)
        colp_sb = res.tile([128, jt_n], fp32)
        ldiag_sb = res.tile([128, it_n], fp32)
        rows_sb = res.tile([1, ch], fp32)
        ident = res.tile([128, 128], fp32)
        # dual-fp8 ldweights needs the dual-row byte stride 16-aligned
        ones8 = res.tile([128, 2, 16], f8)

        make_identity(nc, ident)
        nc.vector.memset(ones8, 1.0)

        # ---- loads: e1t (small), e2t in j-groups so compute starts early ----
        for k in range(kt):
            nc.sync.dma_start(out=e1t_sb[:, k, :], in_=e1t_d[ts(k, 128), :])
        jg_n = 4
        jgw = n // jg_n
        for jg in range(jg_n):
            for k in range(kt):
                nc.sync.dma_start(out=e2t_sb[:, k, ts(jg, jgw)],
                                  in_=e2t_d[ts(k, 128), ts(jg, jgw)])

        # ---- main loop: 128-row j blocks of the transposed logit tile ----
        for jt in range(jt_n):
            pl = pml.tile([128, ch], fp32, tag="pl")
            for k2 in range(0, kt, 2):
                nc.tensor.matmul(pl, lhsT=e2t_sb[:, k2:k2 + 2, ts(jt, 128)],
                                 rhs=e1t_sb[:, k2:k2 + 2, :],
                                 start=(k2 == 0), stop=(k2 == kt - 2),
                                 perf_mode=PM)
            if jt < it_n:
                # diagonal logits live at [p, jt*128+p] of this tile
                dtmp = junkp.tile([128, 128], fp32, tag="dtmp")
                nc.vector.tensor_mul(out=dtmp, in0=pl[:, ts(jt, 128)],
                                     in1=ident)
                nc.vector.reduce_sum(out=ldiag_sb[:, jt:jt + 1], in_=dtmp,
                                     axis=mybir.AxisListType.X)
            nc.scalar.activation(out=exps_sb[:, jt, :], in_=pl, func=AF.Exp,
                                 scale=INV_T, accum_out=colp_sb[:, jt:jt + 1])

        # ---- row sums: ones^T @ exps, accumulated over all j tile pairs ----
        prow_t = prow.tile([1, ch], fp32)
        for jt in range(0, jt_n, 2):
            nc.tensor.matmul(prow_t, lhsT=ones8[:, :, 0:1],
                             rhs=exps_sb[:, jt:jt + 2, :],
                             start=(jt == 0), stop=(jt == jt_n - 2),
                             perf_mode=PM)
        nc.scalar.copy(out=rows_sb, in_=prow_t)

        nc.sync.dma_start(out=rows_d[:, :], in_=rows_sb)
        nc.sync.dma_start(out=colp_d[:, :], in_=colp_sb)
        nc.sync.dma_start(out=ldiag_d[:, :], in_=ldiag_sb)
    return _legalize_waits(nc) if legalize else nc


def _get_nc():
    if "nc" not in _CACHE:
        _CACHE["nc"] = build_nc()
    return _CACHE["nc"]


def _prep_inputs(embeddings1, embeddings2):
    e1 = np.asarray(embeddings1, dtype=np.float32)
    e2 = np.asarray(embeddings2, dtype=np.float32)
    assert e1.shape == (N, D) and e2.shape == (N, D)
    e1n = e1 / np.maximum(np.linalg.norm(e1, axis=1, keepdims=True), 1e-12)
    e2n = e2 / np.maximum(np.linalg.norm(e2, axis=1, keepdims=True), 1e-12)
    e2t = e2n.T.astype(FP8)  # [D, N]
    in_maps = []
    for c in range(NCORES):
        sl = slice(c * CH, (c + 1) * CH)
        in_maps.append({
            "e1t": np.ascontiguousarray(e1n[sl].T).astype(FP8),
            "e2t": np.ascontiguousarray(np.roll(e2t, -c * CH, axis=1)),
        })
    return in_maps


def _run(in_maps, trace=False, **kw):
    from concourse.bass_utils import run_bass_kernel_spmd
    return run_bass_kernel_spmd(_get_nc(), in_maps,
                                core_ids=list(range(NCORES)),
                                trace=trace, **kw)


def _combine(outs):
    ldiag = np.concatenate(
        [np.asarray(o["ldiag"], dtype=np.float64).T.reshape(-1) for o in outs])
    ldiag *= INV_T
    rows = np.concatenate(
        [np.asarray(o["rows"], dtype=np.float64).reshape(-1) for o in outs])
    colsum = np.zeros(N, dtype=np.float64)
    for c, o in enumerate(outs):
        colsum += np.roll(
            np.asarray(o["colp"], dtype=np.float64).T.reshape(-1), c * CH)

    ed = np.exp(ldiag)
    row_denom = rows - ed
    col_denom = colsum - ed
    sim12 = float(np.sum(ldiag - np.log(row_denom)))
    sim21 = float(np.sum(ldiag - np.log(col_denom)))
    return (np.float32(-sim12), np.float32(-sim21))


def kernel(embeddings1, embeddings2, _trace=False, _full_result=False):
    in_maps = _prep_inputs(embeddings1, embeddings2)
    bres = _run(in_maps, trace=_trace)
    result = _combine(bres.results)
    if _full_result:
        return result, bres
    return result


# revision 7
# speedup vs baseline: 2.6750x; 1.0550x over previous
"""Contrastive loss kernel for Trainium2 (8 NeuronCores, Bass/Tile).

Strategy (data-parallel over rows of embeddings1, fp8 DoubleRow matmul):
  - Host L2-normalizes both embedding sets and ships fp8_e4m3 transposed
    operands: e1t (core's own 512 rows of e1, d-major, k-blocked) and
    e2t (all of e2, d-major, ROTATED by c*CH columns so each core's
    diagonal block lands in its first 4 j-tiles -- keeps the kernel
    SPMD-identical across cores).  e2t is additionally blocked by
    j-group so each group is one contiguous-per-partition 1 MB DMA.
  - Device: 32 j-tiles of the transposed logit tile S_T[j, i] (j on
    partitions, 512 i's on free) via fp8 DoubleRow matmuls (2 k-subtiles
    per instruction, 2x PE throughput vs bf16).
  - ACT applies exp(10*s) -> exps (fp8) with accum_out -> per-j column
    partials; PE ones-matmul over fp8 exps (interleaved with the main
    loop) -> per-i row sums; DVE extracts the diagonal logits from the
    first 4 PSUM tiles (identity-mask multiply + reduce).
  - Host combines: un-rotate column partials, ed = exp(10*ldiag),
    row/col denominators, logs, two scalar sums.

Outputs per core: cold [128,36] = colp [128,32] (column partials,
local j = jt*128+p) ++ ldiag [128,4] (diag cosines, i = it*128+p),
rows [1,512] (row sums incl. diagonal).
"""

import os
import sys

import numpy as np

for _p in ("/root/.axon_site", "/root/.axon_site/_ro/trn_rl_repo",
           "/root/.axon_site/_ro/pypackages", "/opt/trn_rl_repo"):
    if os.path.isdir(_p) and _p not in sys.path:
        sys.path.append(_p)

import ml_dtypes

N, D = 4096, 1024
NCORES = 8
CH = N // NCORES          # 512 rows of e1 per core
INV_T = 10.0              # 1 / temperature
FP8 = ml_dtypes.float8_e4m3
JG = 4                    # e2t DMA j-groups
JGW = N // JG             # 1024 j's per group, 8 j-tiles

_CACHE = {}


def _legalize_waits(nc, cap=1):
    """Split >cap semaphore waits per instruction onto preceding NOPs.

    The walrus build in this container rejects instructions carrying more
    than ~2 sync waits ("Too many sync wait commands"); Tile emits up to
    12 on the final barrier drain.  Hoisting the excess waits onto NOPs
    issued just before, on the same engine queue, is semantics-preserving
    (the engine is in-order, so waiting earlier is safe).
    """
    import concourse.mybir as mybir
    nid = 0
    for f in nc.m.functions:
        for b in f.blocks:
            insts = b.instructions
            i = 0
            while i < len(insts):
                inst = insts[i]
                si = inst.sync_info
                if si is not None and si.on_wait and len(si.on_wait) > cap:
                    waits = list(si.on_wait)
                    inst.sync_info = mybir.SyncInfo(
                        on_wait=waits[-cap:], on_update=list(si.on_update))
                    excess = waits[:-cap]
                    pos = i
                    for j in range(0, len(excess), cap):
                        nop = mybir.InstNoOp(
                            name=f"I-waitnop-{nid}", ins=[], outs=[])
                        nid += 1
                        nop.engine = inst.engine
                        nop.sync_info = mybir.SyncInfo(
                            on_wait=excess[j:j + cap], on_update=[])
                        insts.insert(pos, nop)
                        pos += 1
                        i += 1
                i += 1
    return nc


def build_nc(n=N, d=D, ch=CH, legalize=True):
    import concourse.bass as bass
    import concourse.mybir as mybir
    import concourse.tile as tile
    from concourse.masks import make_identity
    from contextlib import ExitStack

    fp32 = mybir.dt.float32
    f8 = mybir.dt.float8e4
    AF = mybir.ActivationFunctionType
    PM = mybir.MatmulPerfMode.DoubleRow
    ts = bass.ts

    kt = d // 128             # 8 contraction subtiles
    jt_n = n // 128           # 32 j tiles
    it_n = ch // 128          # 4 i tiles
    jtg = JGW // 128          # 8 j tiles per DMA group

    nc = bass.Bass(trn_type="TRN2")
    # blocked layouts: per-partition data contiguous per DMA (8KB lines)
    e1t_d = nc.dram_tensor("e1t", [128, kt * ch], f8, kind="ExternalInput")
    e2t_d = nc.dram_tensor("e2t", [128, JG * kt * JGW], f8,
                           kind="ExternalInput")
    cold_d = nc.dram_tensor("cold", [128, jt_n + it_n], fp32,
                            kind="ExternalOutput")
    rows_d = nc.dram_tensor("rows", [1, ch], fp32, kind="ExternalOutput")

    with ExitStack() as ctx:
        tc = ctx.enter_context(tile.TileContext(nc))
        res = ctx.enter_context(tc.tile_pool(name="res", bufs=1))
        junkp = ctx.enter_context(tc.tile_pool(name="junkp", bufs=2))
        pml = ctx.enter_context(tc.tile_pool(name="pml", bufs=6, space="PSUM"))
        prow = ctx.enter_context(tc.tile_pool(name="prow", bufs=1, space="PSUM"))

        # resident SBUF tensors
        e2g = [res.tile([128, kt, JGW], f8, name=f"e2g{g}")
               for g in range(JG)]
        e1t_sb = res.tile([128, kt, ch], f8)      # e1n^T
        exps_sb = res.tile([128, jt_n, ch], f8)   # exp(logits^T)
        cold_sb = res.tile([128, jt_n + it_n], fp32)
        rows_sb = res.tile([1, ch], fp32)
        ident = res.tile([128, 128], fp32)
        # dual-fp8 ldweights needs the dual-row byte stride 16-aligned
        ones8 = res.tile([128, 2, 16], f8)

        make_identity(nc, ident)
        nc.vector.memset(ones8, 1.0)

        # ---- loads: one DMA per e2t j-group + one for e1t ----
        nc.sync.dma_start(out=e1t_sb, in_=e1t_d[:, :])
        for g in range(JG):
            nc.sync.dma_start(out=e2g[g],
                              in_=e2t_d[:, ts(g, kt * JGW)])

        def rowsum_pair(p):
            # ones^T @ exps for j tiles (2p, 2p+1), accumulated in PSUM
            nc.tensor.matmul(prow_t, lhsT=ones8[:, :, 0:1],
                             rhs=exps_sb[:, 2 * p:2 * p + 2, :],
                             start=(p == 0), stop=(p == jt_n // 2 - 1),
                             perf_mode=PM)

        prow_t = prow.tile([1, ch], fp32)
        # ---- main loop: 128-row j blocks of the transposed logit tile ----
        for jt in range(jt_n):
            g, jl = jt // jtg, jt % jtg
            pl = pml.tile([128, ch], fp32, tag="pl")
            for k2 in range(0, kt, 2):
                nc.tensor.matmul(pl, lhsT=e2g[g][:, k2:k2 + 2, ts(jl, 128)],
                                 rhs=e1t_sb[:, k2:k2 + 2, :],
                                 start=(k2 == 0), stop=(k2 == kt - 2),
                                 perf_mode=PM)
            if jt < it_n:
                # diagonal logits live at [p, jt*128+p] of this tile
                dtmp = junkp.tile([128, 128], fp32, tag="dtmp")
                nc.vector.tensor_mul(out=dtmp, in0=pl[:, ts(jt, 128)],
                                     in1=ident)
                nc.vector.reduce_sum(out=cold_sb[:, jt_n + jt:jt_n + jt + 1],
                                     in_=dtmp, axis=mybir.AxisListType.X)
            nc.scalar.activation(out=exps_sb[:, jt, :], in_=pl, func=AF.Exp,
                                 scale=INV_T,
                                 accum_out=cold_sb[:, jt:jt + 1])
            # interleave row-sum pairs to avoid a serial tail on the PE
            if jt >= 3 and jt % 2 == 1:
                rowsum_pair((jt - 3) // 2)
        rowsum_pair(jt_n // 2 - 1)
        nc.scalar.copy(out=rows_sb, in_=prow_t)

        nc.sync.dma_start(out=rows_d[:, :], in_=rows_sb)
        nc.sync.dma_start(out=cold_d[:, :], in_=cold_sb)
    return _legalize_waits(nc) if legalize else nc


def _get_nc():
    if "nc" not in _CACHE:
        _CACHE["nc"] = build_nc()
    return _CACHE["nc"]


def _prep_inputs(embeddings1, embeddings2):
    e1 = np.asarray(embeddings1, dtype=np.float32)
    e2 = np.asarray(embeddings2, dtype=np.float32)
    assert e1.shape == (N, D) and e2.shape == (N, D)
    e1n = e1 / np.maximum(np.linalg.norm(e1, axis=1, keepdims=True), 1e-12)
    e2n = e2 / np.maximum(np.linalg.norm(e2, axis=1, keepdims=True), 1e-12)
    e2t = e2n.T.astype(FP8)  # [D, N]
    in_maps = []
    for c in range(NCORES):
        sl = slice(c * CH, (c + 1) * CH)
        # e1t blocked: [128, k, m] with row p = d-subrow, contiguous per p
        e1blk = np.ascontiguousarray(e1n[sl].T).astype(FP8)  # [D, CH]
        e1blk = e1blk.reshape(D // 128, 128, CH).transpose(1, 0, 2)
        # e2t rotated then blocked [128, jg, k, m]
        e2rot = np.roll(e2t, -c * CH, axis=1)  # [D, N]
        e2blk = e2rot.reshape(D // 128, 128, JG, JGW).transpose(1, 2, 0, 3)
        in_maps.append({
            "e1t": np.ascontiguousarray(e1blk).reshape(128, D // 128 * CH),
            "e2t": np.ascontiguousarray(e2blk).reshape(128, JG * D // 128 * JGW),
        })
    return in_maps


def _run(in_maps, trace=False, **kw):
    from concourse.bass_utils import run_bass_kernel_spmd
    return run_bass_kernel_spmd(_get_nc(), in_maps,
                                core_ids=list(range(NCORES)),
                                trace=trace, **kw)


def _combine(outs):
    jt_n = N // 128
    ldiag = np.concatenate(
        [np.asarray(o["cold"][:, jt_n:], dtype=np.float64).T.reshape(-1)
         for o in outs])
    ldiag *= INV_T
    rows = np.concatenate(
        [np.asarray(o["rows"], dtype=np.float64).reshape(-1) for o in outs])
    colsum = np.zeros(N, dtype=np.float64)
    for c, o in enumerate(outs):
        colsum += np.roll(
            np.asarray(o["cold"][:, :jt_n], dtype=np.float64).T.reshape(-1),
            c * CH)

    ed = np.exp(ldiag)
    row_denom = rows - ed
    col_denom = colsum - ed
    sim12 = float(np.sum(ldiag - np.log(row_denom)))
    sim21 = float(np.sum(ldiag - np.log(col_denom)))
    return (np.float32(-sim12), np.float32(-sim21))


def kernel(embeddings1, embeddings2, _trace=False, _full_result=False):
    in_maps = _prep_inputs(embeddings1, embeddings2)
    bres = _run(in_maps, trace=_trace)
    result = _combine(bres.results)
    if _full_result:
        return result, bres
    return result
